# revision 5
# baseline (speedup 1.0000x reference)
"""Trainium2 Bass kernel for the GSC Vanilla SNN problem.

3-layer LIF spiking net, S=101 timesteps, B=2048 batch, data-parallel over
batch across 8 NeuronCores (256 rows per core).

Math (per layer, per step, spikingjelly LIF with tau=2, v_th=1, hard reset):
    a_t   = v_{t-1} + c_t              (c_t = matmul current incl. bias)
    z_t   = 0.5 * (a_t < 2)            in {0, 0.5}
    v_t   = a_t * z_t                  (= a/2 if no spike else 0)
The true spike s = 1 - 2*z is folded into the next layer's weights:
    s @ W + b  ==  z @ (-2W) + (b + colsum(W))
and the bias rides the matmul itself as extra contraction rows:
  - layer 1: x is augmented with two ones-rows (K=122), W1 gets hi/lo bias rows
  - layers 2/3: pad lanes h>=200 of z are identically 0.5, so weight rows
    200/201 carry 2*hi / 2*lo of the bias; rows 202+ are zero.
The readout accumulates R = sum_t z3_t @ Wr_pad in PSUM (pair-interleaved
[12, 512]); host applies  out = base - (2/S) * (R0 + R1)  and log_softmax.

Device layout: hidden padded 200->256, stored as [128 partitions, 2, 256]
(m-chunk, batch). Timesteps are processed in PAIRS so every matmul runs at
N=512 (one full PSUM bank per m-chunk) and weight loads amortize. The z
tiles span a pair: [128, 2(k-chunk), 2(step), 256].

Engines: PE all matmuls (bf16, f32 PSUM); ACT evacuates each pair's PSUM to
bf16 SBUF in one [128,1024] activation per layer; DVE does the adds (TT 2x
bf16) and reset-multiplies; GPSIMD computes the z compares.
"""

import numpy as np
import ml_dtypes

S = 101
D = 120            # C*M input features
DA = 122           # augmented with 2 ones-rows for hi/lo bias
H = 200
HP = 256           # padded hidden
DOUT = 12
NCORES = 8
B_FULL = 2048
BC = B_FULL // NCORES   # 256 batch rows per core
TB = 8                  # x DMA block (timesteps per DMA)

# engine assignment per layer: evac "act" (ACT copies PSUM->bf16 SBUF, adds
# run in DVE 2x mode) or "direct" (DVE adds straight from PSUM at 1x);
# op2 (the z compare) on "gpsimd" or "dve".
CFG = {
    "evac": ("act", "act", "act"),
    "op2": ("gpsimd", "gpsimd", "gpsimd"),
}

_bf16 = ml_dtypes.bfloat16

_BUILD_CACHE = {}


def _build(s_steps, bc, tb, cfg=None):
    """Build + compile the Bass program for one core. Returns nc."""
    import concourse.bacc as bacc
    import concourse.mybir as mybir
    import concourse.tile as tile

    cfg = cfg or CFG
    dt = mybir.dt
    alu = mybir.AluOpType
    P = 128
    B2 = 2 * bc

    nc = bacc.Bacc("TRN2", target_bir_lowering=False, debug=False)

    x_d = nc.dram_tensor("x", [DA, s_steps * bc], dt.bfloat16, kind="ExternalInput")
    w1_d = nc.dram_tensor("w1", [DA, HP], dt.bfloat16, kind="ExternalInput")
    w2_d = nc.dram_tensor("w2", [2, P, HP], dt.bfloat16, kind="ExternalInput")
    w3_d = nc.dram_tensor("w3", [2, P, HP], dt.bfloat16, kind="ExternalInput")
    wr_d = nc.dram_tensor("wr", [2, P, DOUT], dt.bfloat16, kind="ExternalInput")
    out_d = nc.dram_tensor("out", [DOUT, B2], dt.float32, kind="ExternalOutput")

    pairs = [(t, t + 1) if t + 1 < s_steps else (t,) for t in range(0, s_steps, 2)]
    n_pairs = len(pairs)

    with tile.TileContext(nc) as tc:
        with (
            tc.tile_pool(name="const", bufs=1) as constp,
            tc.tile_pool(name="xp", bufs=3) as xp,
            tc.tile_pool(name="state", bufs=1) as statep,
            tc.tile_pool(name="work", bufs=3) as workp,
            tc.tile_pool(name="ps", bufs=1, space="PSUM") as psp,
            tc.tile_pool(name="psr", bufs=1, space="PSUM") as psrp,
        ):
            w1 = constp.tile([DA, HP], dt.bfloat16)
            nc.sync.dma_start(w1[:], w1_d[:])
            w2a = constp.tile([P, HP], dt.bfloat16)
            w2b = constp.tile([P, HP], dt.bfloat16)
            nc.sync.dma_start(w2a[:], w2_d[0])
            nc.sync.dma_start(w2b[:], w2_d[1])
            w3a = constp.tile([P, HP], dt.bfloat16)
            w3b = constp.tile([P, HP], dt.bfloat16)
            nc.sync.dma_start(w3a[:], w3_d[0])
            nc.sync.dma_start(w3b[:], w3_d[1])
            wra = constp.tile([P, DOUT], dt.bfloat16)
            wrb = constp.tile([P, DOUT], dt.bfloat16)
            nc.sync.dma_start(wra[:], wr_d[0])
            nc.sync.dma_start(wrb[:], wr_d[1])

            # states [128, 2(m), 256]
            st = [statep.tile([P, 2, bc], dt.bfloat16, name=f"st{i}") for i in range(3)]
            for t_ in st:
                nc.vector.memset(t_[:], 0.0)

            R = psrp.tile([DOUT, B2], dt.float32)

            w23 = [(w2a, w2b), (w3a, w3b)]
            xb = None
            for pi, ts_ in enumerate(pairs):
                np_ = len(ts_)
                t0 = ts_[0]
                W = np_ * bc  # matmul N: 512 or 256 (tail)
                if t0 % tb == 0:
                    ncols = min(tb, s_steps - t0) * bc
                    xb = xp.tile([DA, tb * bc], dt.bfloat16, name="xb")
                    nc.sync.dma_start(
                        xb[:, 0:ncols], x_d[:, t0 * bc : t0 * bc + ncols]
                    )
                xpair = xb[:, (t0 % tb) * bc : (t0 % tb) * bc + W]

                zprev = None
                for li in range(3):
                    # current pair tile: [128, 2(m), 512] f32 = 2 PSUM banks
                    c = psp.tile([P, 2, B2], dt.float32, name=f"c{li + 1}")
                    if li == 0:
                        for m in range(2):
                            nc.tensor.matmul(
                                c[:, m, 0:W],
                                w1[:, m * P : (m + 1) * P],
                                xpair,
                                start=True,
                                stop=True,
                            )
                    else:
                        wka, wkb = w23[li - 1]
                        for m in range(2):
                            nc.tensor.matmul(
                                c[:, m, 0:W],
                                wka[:, m * P : (m + 1) * P],
                                zprev[:, 0, 0:np_, :],
                                start=True,
                                stop=False,
                            )
                            nc.tensor.matmul(
                                c[:, m, 0:W],
                                wkb[:, m * P : (m + 1) * P],
                                zprev[:, 1, 0:np_, :],
                                start=False,
                                stop=True,
                            )

                    evac_act = cfg["evac"][li] == "act"
                    if evac_act:
                        ch = workp.tile([P, 2, B2], dt.bfloat16, name=f"ch{li + 1}")
                        nc.scalar.copy(ch[:, :, 0:W], c[:, :, 0:W])

                    # z pair tile: [128, 2(k), 2(step), 256]
                    z = workp.tile([P, 2, 2, bc], dt.bfloat16, name=f"z{li + 1}")
                    eng2 = nc.gpsimd if cfg["op2"][li] == "gpsimd" else nc.vector
                    for p in range(np_):
                        a = workp.tile([P, 2, bc], dt.bfloat16, name=f"a{li + 1}")
                        csrc = (ch if evac_act else c)[:, :, p * bc : (p + 1) * bc]
                        nc.vector.tensor_tensor(
                            a[:], st[li][:], csrc, op=alu.add
                        )
                        eng2.tensor_scalar(
                            z[:, :, p, :], a[:], 2.0, 0.5, alu.is_lt, alu.mult
                        )
                        nc.vector.tensor_tensor(
                            st[li][:], a[:], z[:, :, p, :], op=alu.mult
                        )
                    zprev = z

                nc.tensor.matmul(
                    R[:, 0:W], wra[:], zprev[:, 0, 0:np_, :],
                    start=(pi == 0), stop=False, skip_group_check=True,
                )
                nc.tensor.matmul(
                    R[:, 0:W], wrb[:], zprev[:, 1, 0:np_, :],
                    start=False, stop=(pi == n_pairs - 1), skip_group_check=True,
                )

            out_sb = workp.tile([DOUT, B2], dt.float32)
            nc.vector.tensor_copy(out_sb[:], R[:])
            nc.sync.dma_start(out_d[:], out_sb[:])

    nc.compile()
    return nc


def _get_nc(s_steps=S, bc=BC, tb=TB):
    key = (s_steps, bc, tb)
    if key not in _BUILD_CACHE:
        _BUILD_CACHE[key] = _build(s_steps, bc, tb)
    return _BUILD_CACHE[key]


def _hi_lo(v):
    hi = v.astype(_bf16)
    lo = (v - hi.astype(np.float64)).astype(_bf16)
    return hi, lo


def _prep_weights(W1, b1, W2, b2, W3, b3, Wr, br):
    """Host-side weight packing. Returns (device array dict, host affine base)."""
    P = 128

    def pad(w, rows, cols, scale=1.0):
        w = np.asarray(w, np.float64) * scale
        out = np.zeros((rows, cols), np.float64)
        out[: w.shape[0], : w.shape[1]] = w
        return out

    w1p = pad(W1, DA, HP)
    bh = np.zeros(HP, np.float64)
    bh[:H] = np.asarray(b1, np.float64)
    w1p_bf = w1p.astype(_bf16)
    w1p_bf[D], w1p_bf[D + 1] = _hi_lo(bh)

    def mid(W, b):
        wp = pad(W, HP, HP, scale=-2.0).astype(_bf16)
        bh = np.zeros(HP, np.float64)
        bh[:H] = np.asarray(b, np.float64) + np.asarray(W, np.float64).sum(axis=0)
        hi, lo = _hi_lo(bh)
        wp[H] = 2.0 * hi.astype(np.float64)
        wp[H + 1] = 2.0 * lo.astype(np.float64)
        return wp.reshape(2, P, HP)

    w2p = mid(W2, b2)
    w3p = mid(W3, b3)
    wrp = pad(Wr, HP, DOUT).astype(_bf16).reshape(2, P, DOUT)

    base = (np.asarray(br, np.float64) + np.asarray(Wr, np.float64).sum(axis=0)).astype(
        np.float32
    )
    return {"w1": w1p_bf, "w2": w2p, "w3": w3p, "wr": wrp}, base


def _prep_x(x):
    """[B,C,S,M] f32 -> per-core [DA, S*bc] bf16 list (with two ones-rows)."""
    x = np.asarray(x, np.float32)
    B = x.shape[0]
    bc = B // NCORES
    # [C, M, S, B] -> [D, S, B]
    xt = np.ascontiguousarray(x.transpose(1, 3, 2, 0)).reshape(D, S, B).astype(_bf16)
    outs = []
    for i in range(NCORES):
        xc = np.ones((DA, S * bc), dtype=_bf16)
        xc[:D] = xt[:, :, i * bc : (i + 1) * bc].reshape(D, S * bc)
        outs.append(xc)
    return outs


def _postprocess(R_list, base):
    """R per core [12, 2*bc] (pair-interleaved) -> full [B, 12] log_softmax."""
    outs = []
    for R in R_list:
        bc = R.shape[1] // 2
        Rs = (R[:, :bc] + R[:, bc:]).astype(np.float32)
        o = base[None, :] - (2.0 / S) * Rs.T
        m = o.max(axis=1, keepdims=True)
        z = o - m
        lse = np.log(np.exp(z).sum(axis=1, keepdims=True))
        outs.append(z - lse)
    return np.concatenate(outs, axis=0).astype(np.float32)


def _ensure_ntff_hook():
    """Inject antenv.axon_hooks (NTFF profile hook) if the image lacks it."""
    import sys
    try:
        from antenv.axon_hooks import get_axon_ntff_profile_hook  # noqa: F401
        return True
    except ImportError:
        pass
    import contextlib
    import ctypes
    import types

    so_path = "/opt/axon/libaxon_pjrt.so"
    try:
        lib = ctypes.CDLL(so_path)
    except OSError:
        return False
    if not hasattr(lib, "axon_start_nrt_profile"):
        return False
    lib.axon_start_nrt_profile.argtypes = [
        ctypes.POINTER(ctypes.c_int64),
        ctypes.c_size_t,
    ]
    lib.axon_start_nrt_profile.restype = ctypes.c_int64
    lib.axon_stop_nrt_profile.argtypes = [ctypes.c_char_p]
    lib.axon_stop_nrt_profile.restype = ctypes.c_int64

    @contextlib.contextmanager
    def _hook(output_dir, device_ids):
        import jax

        jax.devices()
        if device_ids:
            ids = (ctypes.c_int64 * len(device_ids))(*device_ids)
            rc = lib.axon_start_nrt_profile(ids, len(device_ids))
        else:
            rc = lib.axon_start_nrt_profile(None, 0)
        if rc != 0:
            raise RuntimeError(f"axon_start_nrt_profile rc={rc}")
        try:
            yield
        finally:
            n = lib.axon_stop_nrt_profile(str(output_dir).encode())
            if n < 0:
                raise RuntimeError(f"axon_stop_nrt_profile rc={n}")

    mod = types.ModuleType("antenv.axon_hooks")
    mod._hook = _hook
    mod.get_axon_ntff_profile_hook = lambda: _hook
    mod.set_axon_ntff_profile_hook = lambda h: setattr(mod, "_hook", h)
    import antenv

    sys.modules["antenv.axon_hooks"] = mod
    antenv.axon_hooks = mod
    return True


def kernel(x, W1, b1, W2, b2, W3, b3, Wr, br, _trace=False):
    from concourse.bass_utils import run_bass_kernel_spmd

    if _trace:
        _trace = _ensure_ntff_hook()
    nc = _get_nc()
    wmap, base = _prep_weights(W1, b1, W2, b2, W3, b3, Wr, br)
    xs = _prep_x(x)
    in_maps = [{**wmap, "x": xs[i]} for i in range(NCORES)]
    res = run_bass_kernel_spmd(
        nc, in_maps, core_ids=list(range(NCORES)), trace=_trace
    )
    R_list = [res.results[i]["out"] for i in range(NCORES)]
    out = _postprocess(R_list, base)
    if _trace:
        kernel.last_exec_time_ns = res.exec_time_ns
        kernel.last_results = res
    return out


kernel.last_exec_time_ns = None
kernel.last_results = None


# revision 6
# speedup vs baseline: 7.3333x; 7.3333x over previous
"""Trainium2 Bass kernel for the GSC Vanilla SNN problem.

3-layer LIF spiking net, S=101 timesteps, B=2048 batch, data-parallel over
batch across 8 NeuronCores (256 rows per core).

Math (per layer, per step, spikingjelly LIF with tau=2, v_th=1, hard reset):
    a_t   = v_{t-1} + c_t              (c_t = matmul current incl. bias)
    z_t   = 0.5 * (a_t < 2)            in {0, 0.5}
    v_t   = a_t * z_t                  (= a/2 if no spike else 0)
The true spike s = 1 - 2*z is folded into the next layer's weights:
    s @ W + b  ==  z @ (-2W) + (b + colsum(W))
and the bias rides the matmul itself as extra contraction rows:
  - layer 1: x is augmented with two ones-rows (K=122), W1 gets hi/lo bias rows
  - layers 2/3: pad lanes h>=200 of z are identically 0.5, so weight rows
    200/201 carry 2*hi / 2*lo of the bias; rows 202+ are zero.
The readout accumulates R = sum_t z3_t @ Wr_pad in PSUM (pair-interleaved
[12, 512]); host applies  out = base - (2/S) * (R0 + R1)  and log_softmax.

Device layout: hidden padded 200->256, stored as [128 partitions, 2, 256]
(m-chunk, batch). Timesteps are processed in PAIRS so every matmul runs at
N=512 (one full PSUM bank per m-chunk) and weight loads amortize. The z
tiles span a pair: [128, 2(k-chunk), 2(step), 256].

Engines: PE all matmuls (bf16, f32 PSUM); ACT evacuates each pair's PSUM to
bf16 SBUF in one [128,1024] activation per layer; DVE does the adds (TT 2x
bf16) and reset-multiplies; GPSIMD computes the z compares.
"""

import numpy as np
import ml_dtypes

S = 101
D = 120            # C*M input features
DA = 122           # augmented with 2 ones-rows for hi/lo bias
H = 200
HP = 256           # padded hidden
DOUT = 12
NCORES = 8
B_FULL = 2048
BC = B_FULL // NCORES   # 256 batch rows per core
TB = 8                  # x DMA block (timesteps per DMA)

# engine assignment per layer: evac "act" (ACT copies PSUM->bf16 SBUF, adds
# run in DVE 2x mode) or "direct" (DVE adds straight from PSUM at 1x);
# op2 (the z compare) on "gpsimd" or "dve".
CFG = {
    "evac": ("act", "act", "act"),
    "op2": ("dve", "dve", "dve"),
}

_bf16 = ml_dtypes.bfloat16

_BUILD_CACHE = {}


def _build(s_steps, bc, tb, cfg=None):
    """Build + compile the Bass program for one core. Returns nc."""
    import concourse.bacc as bacc
    import concourse.mybir as mybir
    import concourse.tile as tile

    cfg = cfg or CFG
    dt = mybir.dt
    alu = mybir.AluOpType
    P = 128
    B2 = 2 * bc

    nc = bacc.Bacc("TRN2", target_bir_lowering=False, debug=False)

    x_d = nc.dram_tensor("x", [DA, s_steps * bc], dt.bfloat16, kind="ExternalInput")
    w1_d = nc.dram_tensor("w1", [DA, HP], dt.bfloat16, kind="ExternalInput")
    w2_d = nc.dram_tensor("w2", [2, P, HP], dt.bfloat16, kind="ExternalInput")
    w3_d = nc.dram_tensor("w3", [2, P, HP], dt.bfloat16, kind="ExternalInput")
    wr_d = nc.dram_tensor("wr", [2, P, DOUT], dt.bfloat16, kind="ExternalInput")
    out_d = nc.dram_tensor("out", [DOUT, B2], dt.float32, kind="ExternalOutput")

    pairs = [(t, t + 1) if t + 1 < s_steps else (t,) for t in range(0, s_steps, 2)]
    n_pairs = len(pairs)

    with tile.TileContext(nc) as tc:
        with (
            tc.tile_pool(name="const", bufs=1) as constp,
            tc.tile_pool(name="xp", bufs=3) as xp,
            tc.tile_pool(name="state", bufs=1) as statep,
            tc.tile_pool(name="work", bufs=3) as workp,
            tc.tile_pool(name="ps", bufs=1, space="PSUM") as psp,
            tc.tile_pool(name="psr", bufs=1, space="PSUM") as psrp,
        ):
            w1 = constp.tile([DA, HP], dt.bfloat16)
            nc.sync.dma_start(w1[:], w1_d[:])
            w2a = constp.tile([P, HP], dt.bfloat16)
            w2b = constp.tile([P, HP], dt.bfloat16)
            nc.sync.dma_start(w2a[:], w2_d[0])
            nc.sync.dma_start(w2b[:], w2_d[1])
            w3a = constp.tile([P, HP], dt.bfloat16)
            w3b = constp.tile([P, HP], dt.bfloat16)
            nc.sync.dma_start(w3a[:], w3_d[0])
            nc.sync.dma_start(w3b[:], w3_d[1])
            wra = constp.tile([P, DOUT], dt.bfloat16)
            wrb = constp.tile([P, DOUT], dt.bfloat16)
            nc.sync.dma_start(wra[:], wr_d[0])
            nc.sync.dma_start(wrb[:], wr_d[1])

            # states [128, 2(m), 256]
            st = [statep.tile([P, 2, bc], dt.bfloat16, name=f"st{i}") for i in range(3)]
            for t_ in st:
                nc.vector.memset(t_[:], 0.0)

            R = psrp.tile([DOUT, B2], dt.float32)

            w23 = [(w2a, w2b), (w3a, w3b)]
            xb = None
            for pi, ts_ in enumerate(pairs):
                np_ = len(ts_)
                t0 = ts_[0]
                W = np_ * bc  # matmul N: 512 or 256 (tail)
                if t0 % tb == 0:
                    ncols = min(tb, s_steps - t0) * bc
                    xb = xp.tile([DA, tb * bc], dt.bfloat16, name="xb")
                    nc.sync.dma_start(
                        xb[:, 0:ncols], x_d[:, t0 * bc : t0 * bc + ncols]
                    )
                xpair = xb[:, (t0 % tb) * bc : (t0 % tb) * bc + W]

                zprev = None
                for li in range(3):
                    # current pair tile: [128, 2(m), 512] f32 = 2 PSUM banks
                    c = psp.tile([P, 2, B2], dt.float32, name=f"c{li + 1}")
                    if li == 0:
                        for m in range(2):
                            nc.tensor.matmul(
                                c[:, m, 0:W],
                                w1[:, m * P : (m + 1) * P],
                                xpair,
                                start=True,
                                stop=True,
                            )
                    else:
                        wka, wkb = w23[li - 1]
                        for m in range(2):
                            nc.tensor.matmul(
                                c[:, m, 0:W],
                                wka[:, m * P : (m + 1) * P],
                                zprev[:, 0, 0:np_, :],
                                start=True,
                                stop=False,
                            )
                            nc.tensor.matmul(
                                c[:, m, 0:W],
                                wkb[:, m * P : (m + 1) * P],
                                zprev[:, 1, 0:np_, :],
                                start=False,
                                stop=True,
                            )

                    evac_act = cfg["evac"][li] == "act"
                    if evac_act:
                        ch = workp.tile([P, 2, B2], dt.bfloat16, name=f"ch{li + 1}")
                        nc.scalar.copy(ch[:, :, 0:W], c[:, :, 0:W])

                    # z pair tile: [128, 2(k), 2(step), 256]
                    z = workp.tile([P, 2, 2, bc], dt.bfloat16, name=f"z{li + 1}")
                    eng2 = nc.gpsimd if cfg["op2"][li] == "gpsimd" else nc.vector
                    for p in range(np_):
                        a = workp.tile([P, 2, bc], dt.bfloat16, name=f"a{li + 1}")
                        csrc = (ch if evac_act else c)[:, :, p * bc : (p + 1) * bc]
                        nc.vector.tensor_tensor(
                            a[:], st[li][:], csrc, op=alu.add
                        )
                        eng2.tensor_scalar(
                            z[:, :, p, :], a[:], 2.0, 0.5, alu.is_lt, alu.mult
                        )
                        nc.vector.tensor_tensor(
                            st[li][:], a[:], z[:, :, p, :], op=alu.mult
                        )
                    zprev = z

                nc.tensor.matmul(
                    R[:, 0:W], wra[:], zprev[:, 0, 0:np_, :],
                    start=(pi == 0), stop=False, skip_group_check=True,
                )
                nc.tensor.matmul(
                    R[:, 0:W], wrb[:], zprev[:, 1, 0:np_, :],
                    start=False, stop=(pi == n_pairs - 1), skip_group_check=True,
                )

            out_sb = workp.tile([DOUT, B2], dt.float32)
            nc.vector.tensor_copy(out_sb[:], R[:])
            nc.sync.dma_start(out_d[:], out_sb[:])

    nc.compile()
    return nc


def _get_nc(s_steps=S, bc=BC, tb=TB):
    key = (s_steps, bc, tb)
    if key not in _BUILD_CACHE:
        _BUILD_CACHE[key] = _build(s_steps, bc, tb)
    return _BUILD_CACHE[key]


def _hi_lo(v):
    hi = v.astype(_bf16)
    lo = (v - hi.astype(np.float64)).astype(_bf16)
    return hi, lo


def _prep_weights(W1, b1, W2, b2, W3, b3, Wr, br):
    """Host-side weight packing. Returns (device array dict, host affine base)."""
    P = 128

    def pad(w, rows, cols, scale=1.0):
        w = np.asarray(w, np.float64) * scale
        out = np.zeros((rows, cols), np.float64)
        out[: w.shape[0], : w.shape[1]] = w
        return out

    w1p = pad(W1, DA, HP)
    bh = np.zeros(HP, np.float64)
    bh[:H] = np.asarray(b1, np.float64)
    w1p_bf = w1p.astype(_bf16)
    w1p_bf[D], w1p_bf[D + 1] = _hi_lo(bh)

    def mid(W, b):
        wp = pad(W, HP, HP, scale=-2.0).astype(_bf16)
        bh = np.zeros(HP, np.float64)
        bh[:H] = np.asarray(b, np.float64) + np.asarray(W, np.float64).sum(axis=0)
        hi, lo = _hi_lo(bh)
        wp[H] = 2.0 * hi.astype(np.float64)
        wp[H + 1] = 2.0 * lo.astype(np.float64)
        return wp.reshape(2, P, HP)

    w2p = mid(W2, b2)
    w3p = mid(W3, b3)
    wrp = pad(Wr, HP, DOUT).astype(_bf16).reshape(2, P, DOUT)

    base = (np.asarray(br, np.float64) + np.asarray(Wr, np.float64).sum(axis=0)).astype(
        np.float32
    )
    return {"w1": w1p_bf, "w2": w2p, "w3": w3p, "wr": wrp}, base


def _prep_x(x):
    """[B,C,S,M] f32 -> per-core [DA, S*bc] bf16 list (with two ones-rows)."""
    x = np.asarray(x, np.float32)
    B = x.shape[0]
    bc = B // NCORES
    # [C, M, S, B] -> [D, S, B]
    xt = np.ascontiguousarray(x.transpose(1, 3, 2, 0)).reshape(D, S, B).astype(_bf16)
    outs = []
    for i in range(NCORES):
        xc = np.ones((DA, S * bc), dtype=_bf16)
        xc[:D] = xt[:, :, i * bc : (i + 1) * bc].reshape(D, S * bc)
        outs.append(xc)
    return outs


def _postprocess(R_list, base):
    """R per core [12, 2*bc] (pair-interleaved) -> full [B, 12] log_softmax."""
    outs = []
    for R in R_list:
        bc = R.shape[1] // 2
        Rs = (R[:, :bc] + R[:, bc:]).astype(np.float32)
        o = base[None, :] - (2.0 / S) * Rs.T
        m = o.max(axis=1, keepdims=True)
        z = o - m
        lse = np.log(np.exp(z).sum(axis=1, keepdims=True))
        outs.append(z - lse)
    return np.concatenate(outs, axis=0).astype(np.float32)


def _ensure_ntff_hook():
    """Inject antenv.axon_hooks (NTFF profile hook) if the image lacks it."""
    import sys
    try:
        from antenv.axon_hooks import get_axon_ntff_profile_hook  # noqa: F401
        return True
    except ImportError:
        pass
    import contextlib
    import ctypes
    import types

    so_path = "/opt/axon/libaxon_pjrt.so"
    try:
        lib = ctypes.CDLL(so_path)
    except OSError:
        return False
    if not hasattr(lib, "axon_start_nrt_profile"):
        return False
    lib.axon_start_nrt_profile.argtypes = [
        ctypes.POINTER(ctypes.c_int64),
        ctypes.c_size_t,
    ]
    lib.axon_start_nrt_profile.restype = ctypes.c_int64
    lib.axon_stop_nrt_profile.argtypes = [ctypes.c_char_p]
    lib.axon_stop_nrt_profile.restype = ctypes.c_int64

    @contextlib.contextmanager
    def _hook(output_dir, device_ids):
        import jax

        jax.devices()
        if device_ids:
            ids = (ctypes.c_int64 * len(device_ids))(*device_ids)
            rc = lib.axon_start_nrt_profile(ids, len(device_ids))
        else:
            rc = lib.axon_start_nrt_profile(None, 0)
        if rc != 0:
            raise RuntimeError(f"axon_start_nrt_profile rc={rc}")
        try:
            yield
        finally:
            n = lib.axon_stop_nrt_profile(str(output_dir).encode())
            if n < 0:
                raise RuntimeError(f"axon_stop_nrt_profile rc={n}")

    mod = types.ModuleType("antenv.axon_hooks")
    mod._hook = _hook
    mod.get_axon_ntff_profile_hook = lambda: _hook
    mod.set_axon_ntff_profile_hook = lambda h: setattr(mod, "_hook", h)
    import antenv

    sys.modules["antenv.axon_hooks"] = mod
    antenv.axon_hooks = mod
    return True


def kernel(x, W1, b1, W2, b2, W3, b3, Wr, br, _trace=False):
    from concourse.bass_utils import run_bass_kernel_spmd

    if _trace:
        _trace = _ensure_ntff_hook()
    nc = _get_nc()
    wmap, base = _prep_weights(W1, b1, W2, b2, W3, b3, Wr, br)
    xs = _prep_x(x)
    in_maps = [{**wmap, "x": xs[i]} for i in range(NCORES)]
    res = run_bass_kernel_spmd(
        nc, in_maps, core_ids=list(range(NCORES)), trace=_trace
    )
    R_list = [res.results[i]["out"] for i in range(NCORES)]
    out = _postprocess(R_list, base)
    if _trace:
        kernel.last_exec_time_ns = res.exec_time_ns
        kernel.last_results = res
    return out


kernel.last_exec_time_ns = None
kernel.last_results = None
